# revision 22
# baseline (speedup 1.0000x reference)
"""Top-1 MoE layer (Mistral MLP experts, E=2) on 8 Trainium2 cores.

Strategy (expert-parallel + data-parallel, host does dispatch/combine):
  - Host computes the tiny router (T x E logits, softmax, argmax) in fp64,
    sorts token indices by assigned expert, and splits each expert's tokens
    evenly across that expert's cores (4 cores per expert when balanced).
  - Each core receives: its packed tokens (transposed, bf16, k-tiled), its
    expert's weights pre-tiled so every device DMA is fully contiguous, and
    the routing weight per token (replicated across partitions).
  - Device kernel per core (bf16 matmuls, fp32 PSUM accumulation): FF is
    processed in quarters so each weight byte is streamed from HBM exactly
    once; h = silu(x@Wg^T) * (x@Wu^T) for a quarter stays in SBUF, partial
    down-projections accumulate into an SBUF fp32 y buffer, and the final
    quarter fuses the per-token routing-weight scale. No collectives.
  - Host scatters per-core outputs back to token order.
"""

import math

import numpy as np
import ml_dtypes

B, S, D, FF, E = 4, 2048, 2048, 8192, 2
T = B * S
P = 128
KT = D // P   # 16 contraction tiles for gate/up
FT = FF // P  # 64 f tiles
DT = D // P   # 16 output-row tiles for down
NQ = 4        # FF quarters
FQ = FT // NQ  # 16 f tiles per quarter
N_CORES = 8
MAX_N = 512   # matmul free-dim / PSUM bank limit (fp32 out)

_nc_cache: dict[int, object] = {}

# Last BassKernelResults (for external profiling harnesses).
LAST = None


def _chunks(C):
    n = max(1, math.ceil(C / MAX_N))
    tc = min(MAX_N, ((C + n - 1) // n + 7) // 8 * 8)
    sizes = []
    left = C
    for _ in range(n):
        sizes.append(min(tc, left))
        left -= sizes[-1]
    assert sum(sizes) == C and all(0 < s <= MAX_N for s in sizes)
    return sizes


def _build_nc(C: int):
    """Build + compile the single-core Bass program (SPMD across 8 cores).

    C = per-core token capacity (multiple of 8).
    """
    import concourse.mybir as mybir
    import concourse.tile as tile
    from concourse import bacc

    dt = mybir.dt
    nc = bacc.Bacc("TRN2", target_bir_lowering=False, debug=False,
                   num_devices=N_CORES)

    # xt[p, ki, t] = x_packed[t, ki*128 + p]
    xt_d = nc.dram_tensor("xt", [P, KT, C], dt.bfloat16, kind="ExternalInput")
    # wg[f, p, ki, m] = w_gate[f*128+m, ki*128+p] (one expert)
    wg_d = nc.dram_tensor("wg", [FT, P, KT, P], dt.bfloat16, kind="ExternalInput")
    wu_d = nc.dram_tensor("wu", [FT, P, KT, P], dt.bfloat16, kind="ExternalInput")
    # wd[do, q, p, fl, m] = w_down[do*128+m, (q*FQ+fl)*128+p]
    wd_d = nc.dram_tensor("wd", [DT, NQ, P, FQ, P], dt.bfloat16,
                          kind="ExternalInput")
    # tw[p, t] = routing weight of token t (same for all p)
    tw_d = nc.dram_tensor("tw", [P, C], dt.float32, kind="ExternalInput")
    # y[do, m, t] = out_packed[t, do*128+m]
    y_d = nc.dram_tensor("y", [DT, P, C], dt.float32, kind="ExternalOutput")

    sizes = _chunks(C)
    starts = [sum(sizes[:i]) for i in range(len(sizes))]
    TC = sizes[0]
    # at very large C (heavily skewed routing) the resident x/h/y buffers
    # leave less SBUF headroom — shrink the weight-stream double-buffering
    wbufs = 3 if C <= 1100 else 2

    with tile.TileContext(nc) as tc:
        with (
            tc.tile_pool(name="persist", bufs=1) as pp,
            tc.tile_pool(name="wgwu", bufs=wbufs) as wp,
            tc.tile_pool(name="wdp", bufs=2) as dp,
            tc.tile_pool(name="hbuf", bufs=1) as hp,
            tc.tile_pool(name="stage", bufs=2) as sp,
            tc.tile_pool(name="psum", bufs=2, space="PSUM") as psp,
        ):
            # ---- head: stream the f=0 gate tile per-ki on sync so the
            # first LDWEIGHTS can start after ~32KB, and split chunk 0 of
            # x per-ki across both hardware DGE queues so the first
            # matmul group is fed as the pieces land. A few warm-up
            # matmuls on the first weight slices keep the PE busy (HAM
            # un-throttles) while x streams in. Steady state unchanged.
            wg0 = wp.tile([P, KT, P], dt.bfloat16, tag="wg")
            for ki in range(KT):
                nc.sync.dma_start(out=wg0[:, ki : ki + 1, :],
                                  in_=wg_d[0, :, ki : ki + 1, :])

            warm_ps = psp.tile([P, TC], dt.float32, tag="g")
            for _ in range(4):
                nc.tensor.matmul(warm_ps[:, :P], wg0[:, 0:1, :],
                                 wg0[:, 0:1, :], start=True, stop=True)
            if TC >= 2 * P:
                for _ in range(16):
                    nc.tensor.matmul(warm_ps[:, : 2 * P], wg0[:, 0:1, :],
                                     wg0[:, 0:2, :], start=True, stop=True)

            xt = pp.tile([P, KT, C], dt.bfloat16)
            t0, tn = starts[0], sizes[0]
            for ki in range(KT):
                eng = nc.scalar if ki % 2 == 0 else nc.sync
                eng.dma_start(
                    out=xt[:, ki : ki + 1, t0 : t0 + tn],
                    in_=xt_d[:, ki : ki + 1, t0 : t0 + tn],
                )
            wu0 = wp.tile([P, KT, P], dt.bfloat16, tag="wu")
            nc.sync.dma_start(out=wu0[:], in_=wu_d[0])
            for c, (t0, tn) in enumerate(zip(starts, sizes)):
                if c == 0:
                    continue
                nc.scalar.dma_start(
                    out=xt[:, :, t0 : t0 + tn],
                    in_=xt_d[:, :, t0 : t0 + tn],
                )
            tw = pp.tile([P, C], dt.float32)
            nc.scalar.dma_start(out=tw[:], in_=tw_d[:])
            h = hp.tile([P, FQ, C], dt.bfloat16)
            y_acc = pp.tile([P, DT, C], dt.float32)

            for q in range(NQ):
                # phase A: h[fl] = silu(x @ Wg^T) * (x @ Wu^T) for this quarter
                for fl in range(FQ):
                    f = q * FQ + fl
                    if f == 0:
                        wg_t, wu_t = wg0, wu0
                    else:
                        wg_t = wp.tile([P, KT, P], dt.bfloat16, tag="wg")
                        nc.sync.dma_start(out=wg_t[:], in_=wg_d[f])
                        wu_t = wp.tile([P, KT, P], dt.bfloat16, tag="wu")
                        nc.sync.dma_start(out=wu_t[:], in_=wu_d[f])
                    for c, (t0, tn) in enumerate(zip(starts, sizes)):
                        tsl = slice(t0, t0 + tn)
                        g_ps = psp.tile([P, TC], dt.float32, tag="g")
                        u_ps = psp.tile([P, TC], dt.float32, tag="u")
                        for ki in range(KT):
                            nc.tensor.matmul(
                                g_ps[:, :tn],
                                wg_t[:, ki : ki + 1, :],
                                xt[:, ki : ki + 1, tsl],
                                start=(ki == 0),
                                stop=(ki == KT - 1),
                            )
                        for ki in range(KT):
                            nc.tensor.matmul(
                                u_ps[:, :tn],
                                wu_t[:, ki : ki + 1, :],
                                xt[:, ki : ki + 1, tsl],
                                start=(ki == 0),
                                stop=(ki == KT - 1),
                            )
                        sg = sp.tile([P, TC], dt.float32, tag="sg")
                        nc.scalar.activation(
                            sg[:, :tn], g_ps[:, :tn],
                            mybir.ActivationFunctionType.Silu,
                        )
                        nc.vector.tensor_mul(
                            h[:, fl, tsl], sg[:, :tn], u_ps[:, :tn]
                        )
                # phase B: y_acc += h @ Wd^T (this quarter's partial)
                for do in range(DT):
                    wd_t = dp.tile([P, FQ, P], dt.bfloat16, tag="wd")
                    nc.sync.dma_start(out=wd_t[:], in_=wd_d[do, q])
                    for c, (t0, tn) in enumerate(zip(starts, sizes)):
                        tsl = slice(t0, t0 + tn)
                        y_ps = psp.tile([P, TC], dt.float32, tag="y")
                        for fl in range(FQ):
                            nc.tensor.matmul(
                                y_ps[:, :tn],
                                wd_t[:, fl : fl + 1, :],
                                h[:, fl : fl + 1, tsl],
                                start=(fl == 0),
                                stop=(fl == FQ - 1),
                            )
                        if q == 0:
                            nc.vector.tensor_copy(
                                y_acc[:, do, tsl], y_ps[:, :tn]
                            )
                        else:
                            nc.vector.tensor_add(
                                y_acc[:, do, tsl], y_acc[:, do, tsl],
                                y_ps[:, :tn],
                            )
                        if q == NQ - 1:
                            y_sb = sp.tile([P, TC], dt.float32, tag="yo")
                            nc.vector.tensor_mul(
                                y_sb[:, :tn], y_acc[:, do, tsl], tw[:, tsl]
                            )
                            nc.sync.dma_start(
                                out=y_d[do, :, tsl], in_=y_sb[:, :tn]
                            )

    nc.compile()
    return nc


def _tile_w_in(w_t):
    """[D, FF] (already transposed) -> [FF/P, P, D/P, P] contiguous bf16."""
    # out[f, p, ki, m] = w_t[ki*128+p, f*128+m]
    r = w_t.reshape(KT, P, FT, P).transpose(2, 1, 0, 3)
    return np.ascontiguousarray(r, dtype=ml_dtypes.bfloat16)


def _tile_w_down(w):
    """w_down [D, FF] -> [D/P, NQ, P, FQ, P] contiguous bf16.

    out[do, q, p, fl, m] = w[do*128+m, (q*FQ+fl)*128+p]
    """
    r = w.reshape(DT, P, NQ, FQ, P).transpose(0, 2, 4, 3, 1)
    return np.ascontiguousarray(r, dtype=ml_dtypes.bfloat16)


def kernel(hidden_states, gate_w, w_gate, w_up, w_down):
    from concourse.bass_utils import run_bass_kernel_spmd

    hidden_states = np.asarray(hidden_states)
    gate_w = np.asarray(gate_w)
    w_gate = np.asarray(w_gate)
    w_up = np.asarray(w_up)
    w_down = np.asarray(w_down)

    x = hidden_states.reshape(T, D)

    # --- router (tiny: T x E) on host, fp64 for stable argmax ---
    logits = x.astype(np.float64) @ gate_w.astype(np.float64).T  # [T, E]
    m = logits.max(axis=1, keepdims=True)
    p = np.exp(logits - m)
    p /= p.sum(axis=1, keepdims=True)
    sel = np.argmax(p, axis=1)  # [T]
    top_w = p[np.arange(T), sel].astype(np.float32)  # [T]

    # --- dispatch: split each expert's tokens across its cores ---
    idx_e = [np.nonzero(sel == e)[0] for e in range(E)]
    t0, t1 = len(idx_e[0]), len(idx_e[1])
    # choose cores per expert minimizing the max per-core load
    best = None
    for n0 in range(1, N_CORES):
        n1 = N_CORES - n0
        load = max(math.ceil(t0 / n0) if t0 else 0,
                   math.ceil(t1 / n1) if t1 else 0)
        if best is None or load < best[0]:
            best = (load, n0)
    # pad capacity to a multiple of 8; matmul/DVE free dims and DMA shapes
    # handle arbitrary sizes, so no 128-rounding.
    C = max(P, ((best[0] + 7) // 8) * 8)
    n0 = best[1]
    cores_per_exp = [n0, N_CORES - n0]

    core_expert = []
    core_tok = []
    for e in range(E):
        ids = idx_e[e]
        nce = cores_per_exp[e]
        per = math.ceil(len(ids) / nce) if len(ids) else 0
        for j in range(nce):
            core_expert.append(e)
            core_tok.append(ids[j * per : (j + 1) * per])

    nc = _nc_cache.get(C)
    if nc is None:
        nc = _build_nc(C)
        _nc_cache[C] = nc

    # --- per-expert weight tiling (shared across that expert's cores) ---
    wg_tiled = [_tile_w_in(w_gate[e].T) for e in range(E)]
    wu_tiled = [_tile_w_in(w_up[e].T) for e in range(E)]
    wd_tiled = [_tile_w_down(w_down[e]) for e in range(E)]

    in_maps = []
    for c in range(N_CORES):
        e = core_expert[c]
        ids = core_tok[c]
        n = len(ids)
        xt = np.zeros((P, KT, C), dtype=ml_dtypes.bfloat16)
        if n:
            # xc [n, D] -> [ki, p, t] -> [p, ki, t]
            xc = x[ids].astype(ml_dtypes.bfloat16)
            xt[:, :, :n] = xc.T.reshape(KT, P, n).transpose(1, 0, 2)
        tw = np.zeros((P, C), dtype=np.float32)
        if n:
            tw[:, :n] = top_w[ids][None, :]
        in_maps.append({
            "xt": xt,
            "wg": wg_tiled[e],
            "wu": wu_tiled[e],
            "wd": wd_tiled[e],
            "tw": tw,
        })

    res = run_bass_kernel_spmd(nc, in_maps, list(range(N_CORES)))
    global LAST
    LAST = res

    # --- combine ---
    out = np.zeros((T, D), dtype=np.float32)
    for c in range(N_CORES):
        ids = core_tok[c]
        n = len(ids)
        if not n:
            continue
        y = res.results[c]["y"]  # [DT, P, C]
        out[ids] = y.reshape(D, C)[:, :n].T
    return out.reshape(B, S, D)



# revision 28
# speedup vs baseline: 1.0108x; 1.0108x over previous
"""Top-1 MoE layer (Mistral MLP experts, E=2) on 8 Trainium2 cores.

Strategy (expert-parallel + data-parallel, host does dispatch/combine):
  - Host computes the tiny router (T x E logits, softmax, argmax) in fp64,
    sorts token indices by assigned expert, and splits each expert's tokens
    evenly across that expert's cores (4 cores per expert when balanced).
  - Each core receives: its packed tokens (transposed, bf16, k-tiled), its
    expert's weights pre-tiled so every device DMA is fully contiguous, and
    the routing weight per token (replicated across partitions).
  - Device kernel per core (bf16 matmuls, fp32 PSUM accumulation): FF is
    processed in quarters so each weight byte is streamed from HBM exactly
    once; h = silu(x@Wg^T) * (x@Wu^T) for a quarter stays in SBUF, partial
    down-projections accumulate into an SBUF fp32 y buffer, and the final
    quarter fuses the per-token routing-weight scale. No collectives.
  - Host scatters per-core outputs back to token order.
"""

import math

import numpy as np
import ml_dtypes

B, S, D, FF, E = 4, 2048, 2048, 8192, 2
T = B * S
P = 128
KT = D // P   # 16 contraction tiles for gate/up
FT = FF // P  # 64 f tiles
DT = D // P   # 16 output-row tiles for down
NQ = 4        # FF quarters
FQ = FT // NQ  # 16 f tiles per quarter
N_CORES = 8
MAX_N = 512   # matmul free-dim / PSUM bank limit (fp32 out)

_nc_cache: dict[int, object] = {}

# Last BassKernelResults (for external profiling harnesses).
LAST = None


def _chunks(C):
    n = max(1, math.ceil(C / MAX_N))
    tc = min(MAX_N, ((C + n - 1) // n + 7) // 8 * 8)
    sizes = []
    left = C
    for _ in range(n):
        sizes.append(min(tc, left))
        left -= sizes[-1]
    assert sum(sizes) == C and all(0 < s <= MAX_N for s in sizes)
    return sizes


def _build_nc(C: int):
    """Build + compile the single-core Bass program (SPMD across 8 cores).

    C = per-core token capacity (multiple of 8).
    """
    import concourse.mybir as mybir
    import concourse.tile as tile
    from concourse import bacc

    dt = mybir.dt
    nc = bacc.Bacc("TRN2", target_bir_lowering=False, debug=False,
                   num_devices=N_CORES)

    # xt[p, KT*t0 + ki*tn + j] = x_packed[t0 + j, ki*128 + p] per chunk
    # (t0, tn): chunk-major so every x DMA moves ~KT*tn contiguous bytes
    # per partition instead of tn-sized lines.
    xt_d = nc.dram_tensor("xt", [P, KT * C], dt.bfloat16, kind="ExternalInput")
    # wg[f, p, ki, m] = w_gate[f*128+m, ki*128+p] (one expert)
    wg_d = nc.dram_tensor("wg", [FT, P, KT, P], dt.bfloat16, kind="ExternalInput")
    wu_d = nc.dram_tensor("wu", [FT, P, KT, P], dt.bfloat16, kind="ExternalInput")
    # wd[do, q, p, fl, m] = w_down[do*128+m, (q*FQ+fl)*128+p]
    wd_d = nc.dram_tensor("wd", [DT, NQ, P, FQ, P], dt.bfloat16,
                          kind="ExternalInput")
    # tw[p, t] = routing weight of token t (same for all p)
    tw_d = nc.dram_tensor("tw", [P, C], dt.float32, kind="ExternalInput")
    # y[do, m, t] = out_packed[t, do*128+m]
    y_d = nc.dram_tensor("y", [DT, P, C], dt.float32, kind="ExternalOutput")

    sizes = _chunks(C)
    starts = [sum(sizes[:i]) for i in range(len(sizes))]
    TC = sizes[0]
    # at very large C (heavily skewed routing) the resident x/h/y buffers
    # leave less SBUF headroom — shrink the weight-stream double-buffering
    wbufs = 3 if C <= 1100 else 2

    with tile.TileContext(nc) as tc:
        with (
            tc.tile_pool(name="persist", bufs=1) as pp,
            tc.tile_pool(name="wgwu", bufs=wbufs) as wp,
            tc.tile_pool(name="wdp", bufs=2) as dp,
            tc.tile_pool(name="hbuf", bufs=1) as hp,
            tc.tile_pool(name="stage", bufs=2) as sp,
            tc.tile_pool(name="psum", bufs=2, space="PSUM") as psp,
        ):
            # ---- head (minimal DMA triggers, ~0.6us engine cost each):
            # sync:   wg0 | x chunk0 first half | wu0 | f>=1 weight stream
            # scalar: x chunk0 second half | chunk1 | chunk2 | tw
            # A short PE warm-up on wg0 opens the HAM clock gate while
            # chunk 0 streams in. Steady state identical to before.
            wg0 = wp.tile([P, KT, P], dt.bfloat16, tag="wg")
            nc.sync.dma_start(out=wg0[:], in_=wg_d[0])

            warm_ps = psp.tile([P, TC], dt.float32, tag="g")
            for _ in range(12):
                if TC >= 2 * P:
                    nc.tensor.matmul(warm_ps[:, : 2 * P], wg0[:, 0:1, :],
                                     wg0[:, 0:2, :], start=True, stop=True)
                else:
                    nc.tensor.matmul(warm_ps[:, :P], wg0[:, 0:1, :],
                                     wg0[:, 0:1, :], start=True, stop=True)

            xt = pp.tile([P, KT * C], dt.bfloat16)
            t0h, tnh = starts[0], sizes[0]
            half = KT * t0h + (KT // 2) * tnh
            nc.sync.dma_start(out=xt[:, KT * t0h : half],
                              in_=xt_d[:, KT * t0h : half])
            nc.scalar.dma_start(out=xt[:, half : KT * (t0h + tnh)],
                                in_=xt_d[:, half : KT * (t0h + tnh)])
            wu0 = wp.tile([P, KT, P], dt.bfloat16, tag="wu")
            nc.sync.dma_start(out=wu0[:], in_=wu_d[0])
            for c, (t0, tn) in enumerate(zip(starts, sizes)):
                if c == 0:
                    continue
                nc.scalar.dma_start(
                    out=xt[:, KT * t0 : KT * (t0 + tn)],
                    in_=xt_d[:, KT * t0 : KT * (t0 + tn)],
                )
            tw = pp.tile([P, C], dt.float32)
            nc.scalar.dma_start(out=tw[:], in_=tw_d[:])
            h = hp.tile([P, FQ, C], dt.bfloat16)
            y_acc = pp.tile([P, DT, C], dt.float32)

            def xsl(t0, tn, ki):
                return xt[:, KT * t0 + ki * tn : KT * t0 + (ki + 1) * tn]

            for q in range(NQ):
                # phase A: h[fl] = silu(x @ Wg^T) * (x @ Wu^T) for this quarter
                for fl in range(FQ):
                    f = q * FQ + fl
                    if f == 0:
                        wg_t, wu_t = wg0, wu0
                    else:
                        wg_t = wp.tile([P, KT, P], dt.bfloat16, tag="wg")
                        nc.sync.dma_start(out=wg_t[:], in_=wg_d[f])
                        wu_t = wp.tile([P, KT, P], dt.bfloat16, tag="wu")
                        nc.sync.dma_start(out=wu_t[:], in_=wu_d[f])
                    for c, (t0, tn) in enumerate(zip(starts, sizes)):
                        tsl = slice(t0, t0 + tn)
                        g_ps = psp.tile([P, TC], dt.float32, tag="g")
                        u_ps = psp.tile([P, TC], dt.float32, tag="u")
                        for ki in range(KT):
                            nc.tensor.matmul(
                                g_ps[:, :tn],
                                wg_t[:, ki : ki + 1, :],
                                xsl(t0, tn, ki),
                                start=(ki == 0),
                                stop=(ki == KT - 1),
                            )
                        for ki in range(KT):
                            nc.tensor.matmul(
                                u_ps[:, :tn],
                                wu_t[:, ki : ki + 1, :],
                                xsl(t0, tn, ki),
                                start=(ki == 0),
                                stop=(ki == KT - 1),
                            )
                        sg = sp.tile([P, TC], dt.float32, tag="sg")
                        nc.scalar.activation(
                            sg[:, :tn], g_ps[:, :tn],
                            mybir.ActivationFunctionType.Silu,
                        )
                        nc.vector.tensor_mul(
                            h[:, fl, tsl], sg[:, :tn], u_ps[:, :tn]
                        )
                # phase B: y_acc += h @ Wd^T (this quarter's partial)
                for do in range(DT):
                    wd_t = dp.tile([P, FQ, P], dt.bfloat16, tag="wd")
                    nc.sync.dma_start(out=wd_t[:], in_=wd_d[do, q])
                    for c, (t0, tn) in enumerate(zip(starts, sizes)):
                        tsl = slice(t0, t0 + tn)
                        y_ps = psp.tile([P, TC], dt.float32, tag="y")
                        for fl in range(FQ):
                            nc.tensor.matmul(
                                y_ps[:, :tn],
                                wd_t[:, fl : fl + 1, :],
                                h[:, fl : fl + 1, tsl],
                                start=(fl == 0),
                                stop=(fl == FQ - 1),
                            )
                        if q == 0:
                            nc.vector.tensor_copy(
                                y_acc[:, do, tsl], y_ps[:, :tn]
                            )
                        else:
                            nc.vector.tensor_add(
                                y_acc[:, do, tsl], y_acc[:, do, tsl],
                                y_ps[:, :tn],
                            )
                        if q == NQ - 1:
                            y_sb = sp.tile([P, TC], dt.float32, tag="yo")
                            nc.vector.tensor_mul(
                                y_sb[:, :tn], y_acc[:, do, tsl], tw[:, tsl]
                            )
                            nc.sync.dma_start(
                                out=y_d[do, :, tsl], in_=y_sb[:, :tn]
                            )

    nc.compile()
    return nc


def _tile_w_in(w_t):
    """[D, FF] (already transposed) -> [FF/P, P, D/P, P] contiguous bf16."""
    # out[f, p, ki, m] = w_t[ki*128+p, f*128+m]
    r = w_t.reshape(KT, P, FT, P).transpose(2, 1, 0, 3)
    return np.ascontiguousarray(r, dtype=ml_dtypes.bfloat16)


def _tile_w_down(w):
    """w_down [D, FF] -> [D/P, NQ, P, FQ, P] contiguous bf16.

    out[do, q, p, fl, m] = w[do*128+m, (q*FQ+fl)*128+p]
    """
    r = w.reshape(DT, P, NQ, FQ, P).transpose(0, 2, 4, 3, 1)
    return np.ascontiguousarray(r, dtype=ml_dtypes.bfloat16)


def _pack_x(x_slots, C):
    """x_slots [C, D] fp32 (padded rows zero) -> [P, KT*C] chunk-major bf16."""
    xt = np.zeros((P, KT * C), dtype=ml_dtypes.bfloat16)
    xb = x_slots.astype(ml_dtypes.bfloat16)
    sizes = _chunks(C)
    t0 = 0
    for tn in sizes:
        blk = xb[t0 : t0 + tn].T.reshape(KT, P, tn)  # [ki, p, t]
        xt[:, KT * t0 : KT * (t0 + tn)] = (
            blk.transpose(1, 0, 2).reshape(P, KT * tn)
        )
        t0 += tn
    return xt


def kernel(hidden_states, gate_w, w_gate, w_up, w_down):
    from concourse.bass_utils import run_bass_kernel_spmd

    hidden_states = np.asarray(hidden_states)
    gate_w = np.asarray(gate_w)
    w_gate = np.asarray(w_gate)
    w_up = np.asarray(w_up)
    w_down = np.asarray(w_down)

    x = hidden_states.reshape(T, D)

    # --- router (tiny: T x E) on host, fp64 for stable argmax ---
    logits = x.astype(np.float64) @ gate_w.astype(np.float64).T  # [T, E]
    m = logits.max(axis=1, keepdims=True)
    p = np.exp(logits - m)
    p /= p.sum(axis=1, keepdims=True)
    sel = np.argmax(p, axis=1)  # [T]
    top_w = p[np.arange(T), sel].astype(np.float32)  # [T]

    # --- dispatch: split each expert's tokens across its cores ---
    idx_e = [np.nonzero(sel == e)[0] for e in range(E)]
    t0, t1 = len(idx_e[0]), len(idx_e[1])
    # choose cores per expert minimizing the max per-core load
    best = None
    for n0 in range(1, N_CORES):
        n1 = N_CORES - n0
        load = max(math.ceil(t0 / n0) if t0 else 0,
                   math.ceil(t1 / n1) if t1 else 0)
        if best is None or load < best[0]:
            best = (load, n0)
    # pad capacity to a multiple of 8; matmul/DVE free dims and DMA shapes
    # handle arbitrary sizes, so no 128-rounding.
    C = max(P, ((best[0] + 7) // 8) * 8)
    n0 = best[1]
    cores_per_exp = [n0, N_CORES - n0]

    core_expert = []
    core_tok = []
    for e in range(E):
        ids = idx_e[e]
        nce = cores_per_exp[e]
        per = math.ceil(len(ids) / nce) if len(ids) else 0
        for j in range(nce):
            core_expert.append(e)
            core_tok.append(ids[j * per : (j + 1) * per])

    nc = _nc_cache.get(C)
    if nc is None:
        nc = _build_nc(C)
        _nc_cache[C] = nc

    # --- per-expert weight tiling (shared across that expert's cores) ---
    wg_tiled = [_tile_w_in(w_gate[e].T) for e in range(E)]
    wu_tiled = [_tile_w_in(w_up[e].T) for e in range(E)]
    wd_tiled = [_tile_w_down(w_down[e]) for e in range(E)]

    in_maps = []
    for c in range(N_CORES):
        e = core_expert[c]
        ids = core_tok[c]
        n = len(ids)
        x_slots = np.zeros((C, D), dtype=np.float32)
        tw = np.zeros((P, C), dtype=np.float32)
        if n:
            x_slots[:n] = x[ids]
            tw[:, :n] = top_w[ids][None, :]
        in_maps.append({
            "xt": _pack_x(x_slots, C),
            "wg": wg_tiled[e],
            "wu": wu_tiled[e],
            "wd": wd_tiled[e],
            "tw": tw,
        })

    res = run_bass_kernel_spmd(nc, in_maps, list(range(N_CORES)))
    global LAST
    LAST = res

    # --- combine ---
    out = np.zeros((T, D), dtype=np.float32)
    for c in range(N_CORES):
        ids = core_tok[c]
        n = len(ids)
        if not n:
            continue
        y = res.results[c]["y"]  # [DT, P, C]
        out[ids] = y.reshape(D, C)[:, :n].T
    return out.reshape(B, S, D)



# revision 29
# speedup vs baseline: 1.0114x; 1.0006x over previous
"""Top-1 MoE layer (Mistral MLP experts, E=2) on 8 Trainium2 cores.

Strategy (expert-parallel + data-parallel, host does dispatch/combine):
  - Host computes the tiny router (T x E logits, softmax, argmax) in fp64,
    sorts token indices by assigned expert, and splits each expert's tokens
    evenly across that expert's cores (4 cores per expert when balanced).
  - Each core receives: its packed tokens (transposed, bf16, k-tiled), its
    expert's weights pre-tiled so every device DMA is fully contiguous, and
    the routing weight per token (replicated across partitions).
  - Device kernel per core (bf16 matmuls, fp32 PSUM accumulation): FF is
    processed in quarters so each weight byte is streamed from HBM exactly
    once; h = silu(x@Wg^T) * (x@Wu^T) for a quarter stays in SBUF, partial
    down-projections accumulate into an SBUF fp32 y buffer, and the final
    quarter fuses the per-token routing-weight scale. No collectives.
  - Host scatters per-core outputs back to token order.
"""

import math

import numpy as np
import ml_dtypes

B, S, D, FF, E = 4, 2048, 2048, 8192, 2
T = B * S
P = 128
KT = D // P   # 16 contraction tiles for gate/up
FT = FF // P  # 64 f tiles
DT = D // P   # 16 output-row tiles for down
NQ = 4        # FF quarters
FQ = FT // NQ  # 16 f tiles per quarter
N_CORES = 8
MAX_N = 512   # matmul free-dim / PSUM bank limit (fp32 out)

_nc_cache: dict[int, object] = {}

# Last BassKernelResults (for external profiling harnesses).
LAST = None


def _chunks(C):
    n = max(1, math.ceil(C / MAX_N))
    tc = min(MAX_N, ((C + n - 1) // n + 7) // 8 * 8)
    sizes = []
    left = C
    for _ in range(n):
        sizes.append(min(tc, left))
        left -= sizes[-1]
    assert sum(sizes) == C and all(0 < s <= MAX_N for s in sizes)
    return sizes


def _build_nc(C: int):
    """Build + compile the single-core Bass program (SPMD across 8 cores).

    C = per-core token capacity (multiple of 8).
    """
    import concourse.mybir as mybir
    import concourse.tile as tile
    from concourse import bacc

    dt = mybir.dt
    nc = bacc.Bacc("TRN2", target_bir_lowering=False, debug=False,
                   num_devices=N_CORES)

    # xt[p, KT*t0 + ki*tn + j] = x_packed[t0 + j, ki*128 + p] per chunk
    # (t0, tn): chunk-major so every x DMA moves ~KT*tn contiguous bytes
    # per partition instead of tn-sized lines.
    xt_d = nc.dram_tensor("xt", [P, KT * C], dt.bfloat16, kind="ExternalInput")
    # wg[f, p, ki, m] = w_gate[f*128+m, ki*128+p] (one expert)
    wg_d = nc.dram_tensor("wg", [FT, P, KT, P], dt.bfloat16, kind="ExternalInput")
    wu_d = nc.dram_tensor("wu", [FT, P, KT, P], dt.bfloat16, kind="ExternalInput")
    # wd[do, q, p, fl, m] = w_down[do*128+m, (q*FQ+fl)*128+p]
    wd_d = nc.dram_tensor("wd", [DT, NQ, P, FQ, P], dt.bfloat16,
                          kind="ExternalInput")
    # tw[p, t] = routing weight of token t (same for all p)
    tw_d = nc.dram_tensor("tw", [P, C], dt.float32, kind="ExternalInput")
    # y[do, m, t] = out_packed[t, do*128+m]
    y_d = nc.dram_tensor("y", [DT, P, C], dt.float32, kind="ExternalOutput")

    sizes = _chunks(C)
    starts = [sum(sizes[:i]) for i in range(len(sizes))]
    TC = sizes[0]
    # at very large C (heavily skewed routing) the resident x/h/y buffers
    # leave less SBUF headroom — shrink the weight-stream double-buffering
    wbufs = 3 if C <= 1100 else 2

    with tile.TileContext(nc) as tc:
        with (
            tc.tile_pool(name="persist", bufs=1) as pp,
            tc.tile_pool(name="wgwu", bufs=wbufs) as wp,
            tc.tile_pool(name="wdp", bufs=2) as dp,
            tc.tile_pool(name="hbuf", bufs=1) as hp,
            tc.tile_pool(name="stage", bufs=2) as sp,
            tc.tile_pool(name="psum", bufs=2, space="PSUM") as psp,
        ):
            # ---- head (minimal DMA triggers, ~0.6us engine cost each):
            # sync:   wg0 | x chunk0 first half | wu0 | f>=1 weight stream
            # scalar: x chunk0 second half | chunk1 | chunk2 | tw
            # A short PE warm-up on wg0 opens the HAM clock gate while
            # chunk 0 streams in. Steady state identical to before.
            wg0 = wp.tile([P, KT, P], dt.bfloat16, tag="wg")
            nc.sync.dma_start(out=wg0[:], in_=wg_d[0])

            warm_ps = psp.tile([P, TC], dt.float32, tag="g")
            for _ in range(14):
                if TC >= 2 * P:
                    nc.tensor.matmul(warm_ps[:, : 2 * P], wg0[:, 0:1, :],
                                     wg0[:, 0:2, :], start=True, stop=True)
                else:
                    nc.tensor.matmul(warm_ps[:, :P], wg0[:, 0:1, :],
                                     wg0[:, 0:1, :], start=True, stop=True)

            # chunk 0 of x streams as 4 ki-quarters, alternating queues in
            # consumption order so the first matmul group starts after
            # ~1/4 of the chunk has landed.
            xt = pp.tile([P, KT * C], dt.bfloat16)
            t0h, tnh = starts[0], sizes[0]
            qk = max(1, KT // 4)
            for qi in range(4):
                a = KT * t0h + qi * qk * tnh
                b = KT * t0h + min((qi + 1) * qk, KT) * tnh
                if a >= b:
                    continue
                eng = nc.sync if qi % 2 == 0 else nc.scalar
                eng.dma_start(out=xt[:, a:b], in_=xt_d[:, a:b])
            wu0 = wp.tile([P, KT, P], dt.bfloat16, tag="wu")
            nc.scalar.dma_start(out=wu0[:], in_=wu_d[0])
            for c, (t0, tn) in enumerate(zip(starts, sizes)):
                if c == 0:
                    continue
                nc.scalar.dma_start(
                    out=xt[:, KT * t0 : KT * (t0 + tn)],
                    in_=xt_d[:, KT * t0 : KT * (t0 + tn)],
                )
            tw = pp.tile([P, C], dt.float32)
            nc.scalar.dma_start(out=tw[:], in_=tw_d[:])
            h = hp.tile([P, FQ, C], dt.bfloat16)
            y_acc = pp.tile([P, DT, C], dt.float32)

            def xsl(t0, tn, ki):
                return xt[:, KT * t0 + ki * tn : KT * t0 + (ki + 1) * tn]

            for q in range(NQ):
                # phase A: h[fl] = silu(x @ Wg^T) * (x @ Wu^T) for this quarter
                for fl in range(FQ):
                    f = q * FQ + fl
                    if f == 0:
                        wg_t, wu_t = wg0, wu0
                    else:
                        wg_t = wp.tile([P, KT, P], dt.bfloat16, tag="wg")
                        nc.sync.dma_start(out=wg_t[:], in_=wg_d[f])
                        wu_t = wp.tile([P, KT, P], dt.bfloat16, tag="wu")
                        nc.sync.dma_start(out=wu_t[:], in_=wu_d[f])
                    for c, (t0, tn) in enumerate(zip(starts, sizes)):
                        tsl = slice(t0, t0 + tn)
                        g_ps = psp.tile([P, TC], dt.float32, tag="g")
                        u_ps = psp.tile([P, TC], dt.float32, tag="u")
                        for ki in range(KT):
                            nc.tensor.matmul(
                                g_ps[:, :tn],
                                wg_t[:, ki : ki + 1, :],
                                xsl(t0, tn, ki),
                                start=(ki == 0),
                                stop=(ki == KT - 1),
                            )
                        for ki in range(KT):
                            nc.tensor.matmul(
                                u_ps[:, :tn],
                                wu_t[:, ki : ki + 1, :],
                                xsl(t0, tn, ki),
                                start=(ki == 0),
                                stop=(ki == KT - 1),
                            )
                        sg = sp.tile([P, TC], dt.float32, tag="sg")
                        nc.scalar.activation(
                            sg[:, :tn], g_ps[:, :tn],
                            mybir.ActivationFunctionType.Silu,
                        )
                        nc.vector.tensor_mul(
                            h[:, fl, tsl], sg[:, :tn], u_ps[:, :tn]
                        )
                # phase B: y_acc += h @ Wd^T (this quarter's partial)
                for do in range(DT):
                    wd_t = dp.tile([P, FQ, P], dt.bfloat16, tag="wd")
                    nc.sync.dma_start(out=wd_t[:], in_=wd_d[do, q])
                    for c, (t0, tn) in enumerate(zip(starts, sizes)):
                        tsl = slice(t0, t0 + tn)
                        y_ps = psp.tile([P, TC], dt.float32, tag="y")
                        for fl in range(FQ):
                            nc.tensor.matmul(
                                y_ps[:, :tn],
                                wd_t[:, fl : fl + 1, :],
                                h[:, fl : fl + 1, tsl],
                                start=(fl == 0),
                                stop=(fl == FQ - 1),
                            )
                        if q == 0:
                            nc.vector.tensor_copy(
                                y_acc[:, do, tsl], y_ps[:, :tn]
                            )
                        else:
                            nc.vector.tensor_add(
                                y_acc[:, do, tsl], y_acc[:, do, tsl],
                                y_ps[:, :tn],
                            )
                        if q == NQ - 1:
                            y_sb = sp.tile([P, TC], dt.float32, tag="yo")
                            nc.vector.tensor_mul(
                                y_sb[:, :tn], y_acc[:, do, tsl], tw[:, tsl]
                            )
                            nc.sync.dma_start(
                                out=y_d[do, :, tsl], in_=y_sb[:, :tn]
                            )

    nc.compile()
    return nc


def _tile_w_in(w_t):
    """[D, FF] (already transposed) -> [FF/P, P, D/P, P] contiguous bf16."""
    # out[f, p, ki, m] = w_t[ki*128+p, f*128+m]
    r = w_t.reshape(KT, P, FT, P).transpose(2, 1, 0, 3)
    return np.ascontiguousarray(r, dtype=ml_dtypes.bfloat16)


def _tile_w_down(w):
    """w_down [D, FF] -> [D/P, NQ, P, FQ, P] contiguous bf16.

    out[do, q, p, fl, m] = w[do*128+m, (q*FQ+fl)*128+p]
    """
    r = w.reshape(DT, P, NQ, FQ, P).transpose(0, 2, 4, 3, 1)
    return np.ascontiguousarray(r, dtype=ml_dtypes.bfloat16)


def _pack_x(x_slots, C):
    """x_slots [C, D] fp32 (padded rows zero) -> [P, KT*C] chunk-major bf16."""
    xt = np.zeros((P, KT * C), dtype=ml_dtypes.bfloat16)
    xb = x_slots.astype(ml_dtypes.bfloat16)
    sizes = _chunks(C)
    t0 = 0
    for tn in sizes:
        blk = xb[t0 : t0 + tn].T.reshape(KT, P, tn)  # [ki, p, t]
        xt[:, KT * t0 : KT * (t0 + tn)] = (
            blk.transpose(1, 0, 2).reshape(P, KT * tn)
        )
        t0 += tn
    return xt


def kernel(hidden_states, gate_w, w_gate, w_up, w_down):
    from concourse.bass_utils import run_bass_kernel_spmd

    hidden_states = np.asarray(hidden_states)
    gate_w = np.asarray(gate_w)
    w_gate = np.asarray(w_gate)
    w_up = np.asarray(w_up)
    w_down = np.asarray(w_down)

    x = hidden_states.reshape(T, D)

    # --- router (tiny: T x E) on host, fp64 for stable argmax ---
    logits = x.astype(np.float64) @ gate_w.astype(np.float64).T  # [T, E]
    m = logits.max(axis=1, keepdims=True)
    p = np.exp(logits - m)
    p /= p.sum(axis=1, keepdims=True)
    sel = np.argmax(p, axis=1)  # [T]
    top_w = p[np.arange(T), sel].astype(np.float32)  # [T]

    # --- dispatch: split each expert's tokens across its cores ---
    idx_e = [np.nonzero(sel == e)[0] for e in range(E)]
    t0, t1 = len(idx_e[0]), len(idx_e[1])
    # choose cores per expert minimizing the max per-core load
    best = None
    for n0 in range(1, N_CORES):
        n1 = N_CORES - n0
        load = max(math.ceil(t0 / n0) if t0 else 0,
                   math.ceil(t1 / n1) if t1 else 0)
        if best is None or load < best[0]:
            best = (load, n0)
    # pad capacity to a multiple of 8; matmul/DVE free dims and DMA shapes
    # handle arbitrary sizes, so no 128-rounding.
    C = max(P, ((best[0] + 7) // 8) * 8)
    n0 = best[1]
    cores_per_exp = [n0, N_CORES - n0]

    core_expert = []
    core_tok = []
    for e in range(E):
        ids = idx_e[e]
        nce = cores_per_exp[e]
        per = math.ceil(len(ids) / nce) if len(ids) else 0
        for j in range(nce):
            core_expert.append(e)
            core_tok.append(ids[j * per : (j + 1) * per])

    nc = _nc_cache.get(C)
    if nc is None:
        nc = _build_nc(C)
        _nc_cache[C] = nc

    # --- per-expert weight tiling (shared across that expert's cores) ---
    wg_tiled = [_tile_w_in(w_gate[e].T) for e in range(E)]
    wu_tiled = [_tile_w_in(w_up[e].T) for e in range(E)]
    wd_tiled = [_tile_w_down(w_down[e]) for e in range(E)]

    in_maps = []
    for c in range(N_CORES):
        e = core_expert[c]
        ids = core_tok[c]
        n = len(ids)
        x_slots = np.zeros((C, D), dtype=np.float32)
        tw = np.zeros((P, C), dtype=np.float32)
        if n:
            x_slots[:n] = x[ids]
            tw[:, :n] = top_w[ids][None, :]
        in_maps.append({
            "xt": _pack_x(x_slots, C),
            "wg": wg_tiled[e],
            "wu": wu_tiled[e],
            "wd": wd_tiled[e],
            "tw": tw,
        })

    res = run_bass_kernel_spmd(nc, in_maps, list(range(N_CORES)))
    global LAST
    LAST = res

    # --- combine ---
    out = np.zeros((T, D), dtype=np.float32)
    for c in range(N_CORES):
        ids = core_tok[c]
        n = len(ids)
        if not n:
            continue
        y = res.results[c]["y"]  # [DT, P, C]
        out[ids] = y.reshape(D, C)[:, :n].T
    return out.reshape(B, S, D)



# revision 30
# speedup vs baseline: 1.0136x; 1.0022x over previous
"""Top-1 MoE layer (Mistral MLP experts, E=2) on 8 Trainium2 cores.

Strategy (expert-parallel + data-parallel, host does dispatch/combine):
  - Host computes the tiny router (T x E logits, softmax, argmax) in fp64,
    sorts token indices by assigned expert, and splits each expert's tokens
    evenly across that expert's cores (4 cores per expert when balanced).
  - Each core receives: its packed tokens (transposed, bf16, k-tiled), its
    expert's weights pre-tiled so every device DMA is fully contiguous, and
    the routing weight per token (replicated across partitions).
  - Device kernel per core (bf16 matmuls, fp32 PSUM accumulation): FF is
    processed in quarters so each weight byte is streamed from HBM exactly
    once; h = silu(x@Wg^T) * (x@Wu^T) for a quarter stays in SBUF, partial
    down-projections accumulate into an SBUF fp32 y buffer, and the final
    quarter fuses the per-token routing-weight scale. No collectives.
  - Host scatters per-core outputs back to token order.
"""

import math

import numpy as np
import ml_dtypes

B, S, D, FF, E = 4, 2048, 2048, 8192, 2
T = B * S
P = 128
KT = D // P   # 16 contraction tiles for gate/up
FT = FF // P  # 64 f tiles
DT = D // P   # 16 output-row tiles for down
NQ = 4        # FF quarters
FQ = FT // NQ  # 16 f tiles per quarter
N_CORES = 8
MAX_N = 512   # matmul free-dim / PSUM bank limit (fp32 out)

_nc_cache: dict[int, object] = {}

# Last BassKernelResults (for external profiling harnesses).
LAST = None


def _chunks(C):
    n = max(1, math.ceil(C / MAX_N))
    tc = min(MAX_N, ((C + n - 1) // n + 7) // 8 * 8)
    sizes = []
    left = C
    for _ in range(n):
        sizes.append(min(tc, left))
        left -= sizes[-1]
    assert sum(sizes) == C and all(0 < s <= MAX_N for s in sizes)
    return sizes


def _build_nc(C: int):
    """Build + compile the single-core Bass program (SPMD across 8 cores).

    C = per-core token capacity (multiple of 8).
    """
    import concourse.mybir as mybir
    import concourse.tile as tile
    from concourse import bacc

    dt = mybir.dt
    nc = bacc.Bacc("TRN2", target_bir_lowering=False, debug=False,
                   num_devices=N_CORES)

    # xt[p, KT*t0 + ki*tn + j] = x_packed[t0 + j, ki*128 + p] per chunk
    # (t0, tn): chunk-major so every x DMA moves ~KT*tn contiguous bytes
    # per partition instead of tn-sized lines.
    xt_d = nc.dram_tensor("xt", [P, KT * C], dt.bfloat16, kind="ExternalInput")
    # wg[f, p, ki, m] = w_gate[f*128+m, ki*128+p] (one expert)
    wg_d = nc.dram_tensor("wg", [FT, P, KT, P], dt.bfloat16, kind="ExternalInput")
    wu_d = nc.dram_tensor("wu", [FT, P, KT, P], dt.bfloat16, kind="ExternalInput")
    # wd[do, q, p, fl, m] = w_down[do*128+m, (q*FQ+fl)*128+p]
    wd_d = nc.dram_tensor("wd", [DT, NQ, P, FQ, P], dt.bfloat16,
                          kind="ExternalInput")
    # tw[p, t] = routing weight of token t (same for all p)
    tw_d = nc.dram_tensor("tw", [P, C], dt.float32, kind="ExternalInput")
    # y[do, m, t] = out_packed[t, do*128+m]
    y_d = nc.dram_tensor("y", [DT, P, C], dt.float32, kind="ExternalOutput")

    sizes = _chunks(C)
    starts = [sum(sizes[:i]) for i in range(len(sizes))]
    TC = sizes[0]
    # at very large C (heavily skewed routing) the resident x/h/y buffers
    # leave less SBUF headroom — shrink the weight-stream double-buffering
    wbufs = 3 if C <= 1100 else 2

    with tile.TileContext(nc) as tc:
        with (
            tc.tile_pool(name="persist", bufs=1) as pp,
            tc.tile_pool(name="wgwu", bufs=wbufs) as wp,
            tc.tile_pool(name="wdp", bufs=2) as dp,
            tc.tile_pool(name="hbuf", bufs=1) as hp,
            tc.tile_pool(name="stage", bufs=2) as sp,
            tc.tile_pool(name="psum", bufs=2, space="PSUM") as psp,
        ):
            # ---- head (minimal DMA triggers, ~0.6us engine cost each):
            # sync:   wg0 | x chunk0 first half | wu0 | f>=1 weight stream
            # scalar: x chunk0 second half | chunk1 | chunk2 | tw
            # A short PE warm-up on wg0 opens the HAM clock gate while
            # chunk 0 streams in. Steady state identical to before.
            wg0 = wp.tile([P, KT, P], dt.bfloat16, tag="wg")
            nc.sync.dma_start(out=wg0[:], in_=wg_d[0])

            warm_ps = psp.tile([P, TC], dt.float32, tag="g")
            for _ in range(14):
                if TC >= 2 * P:
                    nc.tensor.matmul(warm_ps[:, : 2 * P], wg0[:, 0:1, :],
                                     wg0[:, 0:2, :], start=True, stop=True)
                else:
                    nc.tensor.matmul(warm_ps[:, :P], wg0[:, 0:1, :],
                                     wg0[:, 0:1, :], start=True, stop=True)

            # chunk 0 of x streams as 4 ki-quarters, alternating queues in
            # consumption order so the first matmul group starts after
            # ~1/4 of the chunk has landed.
            xt = pp.tile([P, KT * C], dt.bfloat16)
            t0h, tnh = starts[0], sizes[0]
            qk = max(1, KT // 4)
            for qi in range(4):
                a = KT * t0h + qi * qk * tnh
                b = KT * t0h + min((qi + 1) * qk, KT) * tnh
                if a >= b:
                    continue
                eng = nc.sync if qi % 2 == 0 else nc.scalar
                eng.dma_start(out=xt[:, a:b], in_=xt_d[:, a:b])
            wu0 = wp.tile([P, KT, P], dt.bfloat16, tag="wu")
            nc.scalar.dma_start(out=wu0[:], in_=wu_d[0])
            for c, (t0, tn) in enumerate(zip(starts, sizes)):
                if c == 0:
                    continue
                if c == 1:
                    # chunk 1 gates the second matmul group; split it
                    # across both hardware queues so it lands in time.
                    mid = KT * t0 + (KT // 2) * tn
                    nc.sync.dma_start(out=xt[:, KT * t0 : mid],
                                      in_=xt_d[:, KT * t0 : mid])
                    nc.scalar.dma_start(out=xt[:, mid : KT * (t0 + tn)],
                                        in_=xt_d[:, mid : KT * (t0 + tn)])
                else:
                    nc.scalar.dma_start(
                        out=xt[:, KT * t0 : KT * (t0 + tn)],
                        in_=xt_d[:, KT * t0 : KT * (t0 + tn)],
                    )
            tw = pp.tile([P, C], dt.float32)
            nc.scalar.dma_start(out=tw[:], in_=tw_d[:])
            h = hp.tile([P, FQ, C], dt.bfloat16)
            y_acc = pp.tile([P, DT, C], dt.float32)

            def xsl(t0, tn, ki):
                return xt[:, KT * t0 + ki * tn : KT * t0 + (ki + 1) * tn]

            for q in range(NQ):
                # phase A: h[fl] = silu(x @ Wg^T) * (x @ Wu^T) for this quarter
                for fl in range(FQ):
                    f = q * FQ + fl
                    if f == 0:
                        wg_t, wu_t = wg0, wu0
                    else:
                        wg_t = wp.tile([P, KT, P], dt.bfloat16, tag="wg")
                        nc.sync.dma_start(out=wg_t[:], in_=wg_d[f])
                        wu_t = wp.tile([P, KT, P], dt.bfloat16, tag="wu")
                        nc.sync.dma_start(out=wu_t[:], in_=wu_d[f])
                    for c, (t0, tn) in enumerate(zip(starts, sizes)):
                        tsl = slice(t0, t0 + tn)
                        g_ps = psp.tile([P, TC], dt.float32, tag="g")
                        u_ps = psp.tile([P, TC], dt.float32, tag="u")
                        for ki in range(KT):
                            nc.tensor.matmul(
                                g_ps[:, :tn],
                                wg_t[:, ki : ki + 1, :],
                                xsl(t0, tn, ki),
                                start=(ki == 0),
                                stop=(ki == KT - 1),
                            )
                        for ki in range(KT):
                            nc.tensor.matmul(
                                u_ps[:, :tn],
                                wu_t[:, ki : ki + 1, :],
                                xsl(t0, tn, ki),
                                start=(ki == 0),
                                stop=(ki == KT - 1),
                            )
                        sg = sp.tile([P, TC], dt.float32, tag="sg")
                        nc.scalar.activation(
                            sg[:, :tn], g_ps[:, :tn],
                            mybir.ActivationFunctionType.Silu,
                        )
                        nc.vector.tensor_mul(
                            h[:, fl, tsl], sg[:, :tn], u_ps[:, :tn]
                        )
                # phase B: y_acc += h @ Wd^T (this quarter's partial)
                for do in range(DT):
                    wd_t = dp.tile([P, FQ, P], dt.bfloat16, tag="wd")
                    nc.sync.dma_start(out=wd_t[:], in_=wd_d[do, q])
                    for c, (t0, tn) in enumerate(zip(starts, sizes)):
                        tsl = slice(t0, t0 + tn)
                        y_ps = psp.tile([P, TC], dt.float32, tag="y")
                        for fl in range(FQ):
                            nc.tensor.matmul(
                                y_ps[:, :tn],
                                wd_t[:, fl : fl + 1, :],
                                h[:, fl : fl + 1, tsl],
                                start=(fl == 0),
                                stop=(fl == FQ - 1),
                            )
                        if q == 0:
                            nc.vector.tensor_copy(
                                y_acc[:, do, tsl], y_ps[:, :tn]
                            )
                        else:
                            nc.vector.tensor_add(
                                y_acc[:, do, tsl], y_acc[:, do, tsl],
                                y_ps[:, :tn],
                            )
                        if q == NQ - 1:
                            y_sb = sp.tile([P, TC], dt.float32, tag="yo")
                            nc.vector.tensor_mul(
                                y_sb[:, :tn], y_acc[:, do, tsl], tw[:, tsl]
                            )
                            nc.sync.dma_start(
                                out=y_d[do, :, tsl], in_=y_sb[:, :tn]
                            )

    nc.compile()
    return nc


def _tile_w_in(w_t):
    """[D, FF] (already transposed) -> [FF/P, P, D/P, P] contiguous bf16."""
    # out[f, p, ki, m] = w_t[ki*128+p, f*128+m]
    r = w_t.reshape(KT, P, FT, P).transpose(2, 1, 0, 3)
    return np.ascontiguousarray(r, dtype=ml_dtypes.bfloat16)


def _tile_w_down(w):
    """w_down [D, FF] -> [D/P, NQ, P, FQ, P] contiguous bf16.

    out[do, q, p, fl, m] = w[do*128+m, (q*FQ+fl)*128+p]
    """
    r = w.reshape(DT, P, NQ, FQ, P).transpose(0, 2, 4, 3, 1)
    return np.ascontiguousarray(r, dtype=ml_dtypes.bfloat16)


def _pack_x(x_slots, C):
    """x_slots [C, D] fp32 (padded rows zero) -> [P, KT*C] chunk-major bf16."""
    xt = np.zeros((P, KT * C), dtype=ml_dtypes.bfloat16)
    xb = x_slots.astype(ml_dtypes.bfloat16)
    sizes = _chunks(C)
    t0 = 0
    for tn in sizes:
        blk = xb[t0 : t0 + tn].T.reshape(KT, P, tn)  # [ki, p, t]
        xt[:, KT * t0 : KT * (t0 + tn)] = (
            blk.transpose(1, 0, 2).reshape(P, KT * tn)
        )
        t0 += tn
    return xt


def kernel(hidden_states, gate_w, w_gate, w_up, w_down):
    from concourse.bass_utils import run_bass_kernel_spmd

    hidden_states = np.asarray(hidden_states)
    gate_w = np.asarray(gate_w)
    w_gate = np.asarray(w_gate)
    w_up = np.asarray(w_up)
    w_down = np.asarray(w_down)

    x = hidden_states.reshape(T, D)

    # --- router (tiny: T x E) on host, fp64 for stable argmax ---
    logits = x.astype(np.float64) @ gate_w.astype(np.float64).T  # [T, E]
    m = logits.max(axis=1, keepdims=True)
    p = np.exp(logits - m)
    p /= p.sum(axis=1, keepdims=True)
    sel = np.argmax(p, axis=1)  # [T]
    top_w = p[np.arange(T), sel].astype(np.float32)  # [T]

    # --- dispatch: split each expert's tokens across its cores ---
    idx_e = [np.nonzero(sel == e)[0] for e in range(E)]
    t0, t1 = len(idx_e[0]), len(idx_e[1])
    # choose cores per expert minimizing the max per-core load
    best = None
    for n0 in range(1, N_CORES):
        n1 = N_CORES - n0
        load = max(math.ceil(t0 / n0) if t0 else 0,
                   math.ceil(t1 / n1) if t1 else 0)
        if best is None or load < best[0]:
            best = (load, n0)
    # pad capacity to a multiple of 8; matmul/DVE free dims and DMA shapes
    # handle arbitrary sizes, so no 128-rounding.
    C = max(P, ((best[0] + 7) // 8) * 8)
    n0 = best[1]
    cores_per_exp = [n0, N_CORES - n0]

    core_expert = []
    core_tok = []
    for e in range(E):
        ids = idx_e[e]
        nce = cores_per_exp[e]
        per = math.ceil(len(ids) / nce) if len(ids) else 0
        for j in range(nce):
            core_expert.append(e)
            core_tok.append(ids[j * per : (j + 1) * per])

    nc = _nc_cache.get(C)
    if nc is None:
        nc = _build_nc(C)
        _nc_cache[C] = nc

    # --- per-expert weight tiling (shared across that expert's cores) ---
    wg_tiled = [_tile_w_in(w_gate[e].T) for e in range(E)]
    wu_tiled = [_tile_w_in(w_up[e].T) for e in range(E)]
    wd_tiled = [_tile_w_down(w_down[e]) for e in range(E)]

    in_maps = []
    for c in range(N_CORES):
        e = core_expert[c]
        ids = core_tok[c]
        n = len(ids)
        x_slots = np.zeros((C, D), dtype=np.float32)
        tw = np.zeros((P, C), dtype=np.float32)
        if n:
            x_slots[:n] = x[ids]
            tw[:, :n] = top_w[ids][None, :]
        in_maps.append({
            "xt": _pack_x(x_slots, C),
            "wg": wg_tiled[e],
            "wu": wu_tiled[e],
            "wd": wd_tiled[e],
            "tw": tw,
        })

    res = run_bass_kernel_spmd(nc, in_maps, list(range(N_CORES)))
    global LAST
    LAST = res

    # --- combine ---
    out = np.zeros((T, D), dtype=np.float32)
    for c in range(N_CORES):
        ids = core_tok[c]
        n = len(ids)
        if not n:
            continue
        y = res.results[c]["y"]  # [DT, P, C]
        out[ids] = y.reshape(D, C)[:, :n].T
    return out.reshape(B, S, D)

